# revision 1
# baseline (speedup 1.0000x reference)
"""Mesh vertex-normals kernel for 8 trn2 NeuronCores (Bass/Tile).

The reference problem: area-weighted vertex normals of a structured
GRID x GRID triangulated mesh (every quad -> 2 triangles), faces given as
an explicit [n_faces, 3] int32 array.

Key algebraic facts exploited here:
  * weighted face normal = unit_normal * area = cross(vb-va, vc-vb) * 0.5
    (the normalization by |cross| cancels against the area factor), and the
    final per-vertex normalization makes even the 0.5 factor irrelevant.
  * for the structured triangulation produced by setup_inputs(), the
    scatter-add over faces collapses into a fixed 2x2 stencil over the
    vertex grid -> no scatter, no gather, no collectives are needed.
    Each core processes a horizontal band of the grid with 1-row halos.
  * edge-duplicated padding (np.pad mode='edge') makes all phantom quads
    outside the grid degenerate (zero cross product), so boundary vertices
    need no special-casing on device.

Per-core device program (all f32, AoS [row, col, xyz] layout, grid rows on
partitions). Engines cannot read an SBUF operand at a nonzero partition
offset, so the two row-neighbor relations are realized as:
  * V / Vdn: the vertex band loaded twice from DRAM, offset by one row.
  * S[s] = P[s+1] + Q[s]: the P row-shift runs on the (otherwise idle)
    tensor engine as a shifted-identity matmul into PSUM.

    HX  = V[:,c+1]-V[:,c]         (row-aligned horizontal edges)
    HXd = Vdn[:,c+1]-Vdn[:,c]     (row+1 horizontal edges)
    VY  = Vdn-V                   (vertical edges)
    d   = HX + VY[:,c+1]          (quad diagonal)
    C1  = cross(HX, VY[:,c+1])    (triangle 1 weighted normal x2)
    C2  = cross(HXd, d)           (triangle 2 weighted normal x2)
    T   = C1 + C2
    P   = T[:,c+1] + C1           (terms needing quad row s+1)
    Q   = T + C2[:,c+1]           (terms needing quad row s)
    S   = shift_down(P) + Q       (shift via PE matmul)
    out = S * (1 / sqrt(sum(S^2) + tiny))

If `faces` does not match the structured triangulation (it always does for
the reference setup_inputs), we fall back to an exact host computation.
"""

import sys

sys.path.insert(0, "/opt/trn_rl_repo")

import numpy as np

GRID = 1449
N_CORES = 8
COL_CHUNK = 510   # output columns per on-chip tile (3*510 f32 per partition)
PSUM_COLS = 170   # columns per PSUM bank chunk (3*170 = 510 f32 <= 512)


# ---------------------------------------------------------------------------
# host-side helpers
# ---------------------------------------------------------------------------

def _is_structured(faces: np.ndarray, grid: int) -> bool:
    n_quads = (grid - 1) * (grid - 1)
    if faces.shape != (2 * n_quads, 3):
        return False
    idx = np.arange(grid * grid, dtype=np.int64).reshape(grid, grid)
    i00 = idx[:-1, :-1].ravel()
    i01 = idx[:-1, 1:].ravel()
    i10 = idx[1:, :-1].ravel()
    i11 = idx[1:, 1:].ravel()
    f = faces
    return (
        np.array_equal(f[:n_quads, 0], i00)
        and np.array_equal(f[:n_quads, 1], i01)
        and np.array_equal(f[:n_quads, 2], i11)
        and np.array_equal(f[n_quads:, 0], i00)
        and np.array_equal(f[n_quads:, 1], i11)
        and np.array_equal(f[n_quads:, 2], i10)
    )


def _host_fallback(vertices: np.ndarray, faces: np.ndarray) -> np.ndarray:
    """Exact replica of the reference for non-structured faces."""
    n_vertices = vertices.shape[0]
    va = vertices[faces[:, 0]]
    vb = vertices[faces[:, 1]]
    vc = vertices[faces[:, 2]]
    cross = np.cross(vb - va, vc - vb).astype(np.float32)
    norm = np.linalg.norm(cross, axis=-1, keepdims=True)
    weighted = (cross / norm) * (norm * 0.5)
    data = np.broadcast_to(weighted[:, None, :], (faces.shape[0], 3, 3)).reshape(-1, 3)
    summed = np.zeros((n_vertices, 3), dtype=np.float32)
    np.add.at(summed, faces.reshape(-1), data)
    norms = np.linalg.norm(summed, axis=-1, keepdims=True)
    return (summed / np.maximum(norms, 1e-10)).astype(np.float32)


def _band_layout(grid: int, n_cores: int):
    """Row-band sharding: core k outputs rows [base*k, base*k + base + 1)."""
    base = (grid - 1) // n_cores
    assert base * n_cores == grid - 1, "grid-1 must divide evenly"
    out_rows = base + 1          # per-core output rows (1-row overlap)
    in_rows = base + 3           # with halo rows (padded coords)
    return base, out_rows, in_rows


def _row_blocks(n_v_rows: int):
    """Split a band's V rows into <=128-partition blocks, overlapping by 2."""
    blocks = []
    r0 = 0
    while r0 < n_v_rows - 2:
        nv = min(128, n_v_rows - r0)
        blocks.append((r0, nv))
        r0 += nv - 2
    return blocks


def _col_chunks(width: int, chunk: int):
    return [(c0, min(chunk, width - c0)) for c0 in range(0, width, chunk)]


# ---------------------------------------------------------------------------
# device program
# ---------------------------------------------------------------------------

DEFAULT_CFG = dict(
    dma_only=False,
    pe_vshift=False,
    pe_p=False,
    body="v2",        # vy from the PE, C2 = cross(d, vy[:, c]) - no vd load
    col_chunk=483,    # output columns per tile (3 even chunks of 1449)
    psum_cols=161,    # columns per PSUM bank chunk
    io_bufs=3,
    wk_bufs=2,
    psum_bufs=4,
    dma_splits=2,     # split loads across DMA queues (2 was measured optimal)
    wk3_tags=("hx", "vy", "dd"),  # triple-buffered early tiles (pipelining)
    # engine per op: 'v' = vector (DVE), 'g' = gpsimd (Pool)
    # dd on DVE shortens the Pool chain (dd feeds the C2 multiplies)
    eng=dict(hx='v', hd='v', vy='v', dd='v', mm1='v', mm2='v', c1s='g',
             c2s='g', t='g', p='v', q='g', s='v', nsq='v', rcp='v', o='v'),
)


class _PoolMux:
    """Route tile allocations to a deeper pool for selected tags."""
    def __init__(self, main, deep, deep_tags):
        self._main, self._deep, self._tags = main, deep, tuple(deep_tags)
    def tile(self, shape, dtype, tag=None):
        pool = self._deep if tag in self._tags else self._main
        return pool.tile(shape, dtype, tag=tag, name=tag)


def _cfg_key(cfg):
    e = cfg["eng"]
    return (cfg.get("dma_only", False), cfg.get("skip_vd", False),
            cfg.get("pe_vshift", False), cfg.get("pe_p", False),
            cfg.get("body", "v1"),
            cfg["col_chunk"], cfg["psum_cols"],
            cfg["io_bufs"], cfg["wk_bufs"],
            cfg["psum_bufs"], cfg.get("dma_splits", 1), cfg.get("dma_splits_o", 1),
            cfg.get("interleave", False),
            tuple(cfg.get("wk3_tags", ())), cfg.get("wk3_bufs", 3),
            tuple(sorted(e.items())))


def _build_program(grid: int, n_cores: int, repeats: int = 1, cfg=None):
    import contextlib

    import concourse.bacc as bacc
    import concourse.tile as tile
    from concourse import mybir
    from concourse.masks import make_identity

    cfg = cfg or DEFAULT_CFG
    f32 = mybir.dt.float32

    base, out_rows, in_rows = _band_layout(grid, n_cores)
    W = grid + 2      # padded columns

    nc = bacc.Bacc()
    vband = nc.dram_tensor("vband", [in_rows, W, 3], f32, kind="ExternalInput")
    oband = nc.dram_tensor("oband", [out_rows, grid, 3], f32, kind="ExternalOutput")

    with tile.TileContext(nc) as tc:
        with (
            tc.tile_pool(name="io", bufs=cfg["io_bufs"]) as io,
            tc.tile_pool(name="wk", bufs=cfg["wk_bufs"]) as wk,
            tc.tile_pool(name="wk3", bufs=cfg.get("wk3_bufs", 3)) as wk3,
            tc.tile_pool(name="ps", bufs=cfg["psum_bufs"], space="PSUM") as psp,
            tc.tile_pool(name="cst", bufs=1) as cst,
        ):
            wk = _PoolMux(wk, wk3, cfg.get("wk3_tags", ()))
            eps_tile = cst.tile([128, 1], f32, tag="eps")
            nc.vector.memset(eps_tile[:, :], 1e-30)
            # tid[:, :128] = I; columns 128/129 zero => tid[:, 1:129] is the
            # down-shift matrix SH[k, m] = 1 iff k == m+1.
            tid = cst.tile([128, 130], f32, tag="tid")
            nc.gpsimd.memset(tid[:, :], 0.0)
            make_identity(nc, tid[:, 0:128], nomemset=True)
            # tmix[:, 1:129][k, m] = +1 if k == m+1 else (-1 if k == m else 0)
            tmix = cst.tile([128, 130], f32, tag="tmix")
            nc.gpsimd.memset(tmix[:, :], 0.0)
            make_identity(nc, tmix[:, 0:128], nomemset=True)
            nc.gpsimd.affine_select(
                out=tmix[:, 1:129], in_=tmix[:, 1:129],
                compare_op=mybir.AluOpType.not_equal, fill=-1.0, base=0,
                pattern=[[-1, 128]], channel_multiplier=1,
            )

            loop = tc.For_i(0, repeats, 1) if repeats > 1 else contextlib.nullcontext()
            with loop:
                if cfg.get("body", "v1") == "v2":
                    _emit_body_v2(nc, io, wk, psp, eps_tile, tid, tmix,
                                  vband, oband, grid, in_rows, mybir, cfg)
                else:
                    _emit_body(nc, io, wk, psp, eps_tile, tid,
                               vband, oband, grid, in_rows, mybir, cfg)

    nc.finalize()
    return nc


def _emit_body(nc, io, wk, psp, eps_tile, tid, vband, oband,
               grid, in_rows, mybir, cfg):
    f32 = mybir.dt.float32
    Alu = mybir.AluOpType
    Act = mybir.ActivationFunctionType
    ENG = {"v": nc.vector, "g": nc.gpsimd}
    eng = {k: ENG[v] for k, v in cfg["eng"].items()}

    def tt(engine, out, in0, in1, op):
        engine.tensor_tensor(out=out, in0=in0, in1=in1, op=op)

    for r0, nv in _row_blocks(in_rows):
        nq = nv - 1   # quad rows in this block
        ns = nv - 2   # output rows in this block
        for c0, w in _col_chunks(grid, cfg["col_chunk"]):
            # loads (second one shifted down a row)
            v = io.tile([nv, w + 2, 3], f32, tag="v")
            nsp = cfg.get("dma_splits", 1)
            if nsp <= 1:
                nc.sync.dma_start(
                    out=v[:, :, :], in_=vband[r0 : r0 + nv, c0 : c0 + w + 2, :]
                )
            else:
                step = (nv + nsp - 1) // nsp
                for p0 in range(0, nv, step):
                    p1 = min(p0 + step, nv)
                    nc.sync.dma_start(
                        out=v[p0:p1, :, :],
                        in_=vband[r0 + p0 : r0 + p1, c0 : c0 + w + 2, :],
                    )
            vd = io.tile([nq, w + 2, 3], f32, tag="vd")
            if cfg.get("pe_vshift", False):
                # vd = shift_down(v) on the tensor engine; ACT copies to SBUF
                for j0, pw in _col_chunks(w + 2, cfg["psum_cols"]):
                    psv = psp.tile([128, pw, 3], f32, tag="psv")
                    nc.tensor.matmul(
                        out=psv[:, :, :], lhsT=tid[0:nv, 1:129],
                        rhs=v[:, j0 : j0 + pw, :], start=True, stop=True,
                    )
                    nc.scalar.activation(
                        out=vd[:, j0 : j0 + pw, :], in_=psv[0:nq, :, :],
                        func=mybir.ActivationFunctionType.Copy,
                    )
            elif not cfg.get("skip_vd", False):
                nc.sync.dma_start(
                    out=vd[:, :, :],
                    in_=vband[r0 + 1 : r0 + nv, c0 : c0 + w + 2, :],
                )
            if cfg.get("dma_only", False):
                o = io.tile([ns, w, 3], f32, tag="o")
                src2 = v if cfg.get("skip_vd", False) else vd
                nc.vector.tensor_tensor(out=o[:, :, :], in0=v[0:ns, 0:w, :],
                                        in1=src2[0:ns, 0:w, :],
                                        op=mybir.AluOpType.add)
                nc.sync.dma_start(
                    out=oband[r0 : r0 + ns, c0 : c0 + w, :], in_=o[:, :, :]
                )
                continue

            # edge fields
            hx = wk.tile([nq, w + 1, 3], f32, tag="hx")
            tt(eng["hx"], hx[:, :, :], v[0:nq, 1 : w + 2, :],
               v[0:nq, 0 : w + 1, :], Alu.subtract)
            hd = wk.tile([nq, w + 1, 3], f32, tag="hd")
            tt(eng["hd"], hd[:, :, :], vd[:, 1 : w + 2, :],
               vd[:, 0 : w + 1, :], Alu.subtract)
            vy = wk.tile([nq, w + 2, 3], f32, tag="vy")
            tt(eng["vy"], vy[:, :, :], vd[:, :, :], v[0:nq, :, :], Alu.subtract)
            dd = wk.tile([nq, w + 1, 3], f32, tag="dd")
            tt(eng["dd"], dd[:, :, :], hx[:, :, :], vy[:, 1 : w + 2, :], Alu.add)

            # cross products: C1 = hx x vy(c+1),  C2 = hd x d
            m1 = wk.tile([nq, w + 1, 3], f32, tag="m1")
            m2 = wk.tile([nq, w + 1, 3], f32, tag="m2")
            c1 = wk.tile([nq, w + 1, 3], f32, tag="c1")
            c2 = wk.tile([nq, w + 1, 3], f32, tag="c2")
            for k in range(3):
                u, x = (k + 1) % 3, (k + 2) % 3
                tt(eng["mm1"], m1[:, :, k : k + 1], hx[:, :, u : u + 1],
                   vy[:, 1 : w + 2, x : x + 1], Alu.mult)
                tt(eng["mm2"], m2[:, :, k : k + 1], hx[:, :, x : x + 1],
                   vy[:, 1 : w + 2, u : u + 1], Alu.mult)
            tt(eng["c1s"], c1[:, :, :], m1[:, :, :], m2[:, :, :], Alu.subtract)
            for k in range(3):
                u, x = (k + 1) % 3, (k + 2) % 3
                tt(eng["mm1"], m1[:, :, k : k + 1], hd[:, :, u : u + 1],
                   dd[:, :, x : x + 1], Alu.mult)
                tt(eng["mm2"], m2[:, :, k : k + 1], hd[:, :, x : x + 1],
                   dd[:, :, u : u + 1], Alu.mult)
            tt(eng["c2s"], c2[:, :, :], m1[:, :, :], m2[:, :, :], Alu.subtract)

            # T = C1 + C2; P/Q split of the stencil by quad row parity
            t = wk.tile([nq, w + 1, 3], f32, tag="t")
            tt(eng["t"], t[:, :, :], c1[:, :, :], c2[:, :, :], Alu.add)
            pe_p = cfg.get("pe_p", False)
            if not pe_p:
                p = wk.tile([nq, w, 3], f32, tag="p")
                tt(eng["p"], p[:, :, :], t[:, 1 : w + 1, :], c1[:, 0:w, :], Alu.add)
            q = wk.tile([nq, w, 3], f32, tag="q")
            tt(eng["q"], q[:, :, :], t[:, 0:w, :], c2[:, 1 : w + 1, :], Alu.add)

            # S = shift_down(P) + Q  (shift on the tensor engine)
            s = wk.tile([ns, w, 3], f32, tag="s")
            for j0, pw in _col_chunks(w, cfg["psum_cols"]):
                ps = psp.tile([128, pw, 3], f32, tag="ps")
                if pe_p:
                    nc.tensor.matmul(
                        out=ps[:, :, :], lhsT=tid[0:nq, 1:129],
                        rhs=t[:, 1 + j0 : 1 + j0 + pw, :], start=True, stop=False,
                    )
                    nc.tensor.matmul(
                        out=ps[:, :, :], lhsT=tid[0:nq, 1:129],
                        rhs=c1[:, j0 : j0 + pw, :], start=False, stop=True,
                    )
                else:
                    nc.tensor.matmul(
                        out=ps[:, :, :], lhsT=tid[0:nq, 1:129],
                        rhs=p[:, j0 : j0 + pw, :], start=True, stop=True,
                    )
                tt(eng["s"], s[:, j0 : j0 + pw, :], ps[0:ns, :, :],
                   q[0:ns, j0 : j0 + pw, :], Alu.add)

            # normalization
            sq = wk.tile([ns, w, 3], f32, tag="m1")
            nc.scalar.activation(out=sq[:, :, :], in_=s[:, :, :], func=Act.Square)
            nsq = wk.tile([ns, w, 1], f32, tag="nsq")
            tt(eng["nsq"], nsq[:, :, :], sq[:, :, 0:1], sq[:, :, 1:2], Alu.add)
            tt(eng["nsq"], nsq[:, :, :], nsq[:, :, :], sq[:, :, 2:3], Alu.add)
            rn = wk.tile([ns, w, 1], f32, tag="rn")
            nc.scalar.activation(
                out=rn[:, :, :], in_=nsq[:, :, :], func=Act.Sqrt,
                bias=eps_tile[:ns, :],
            )
            if cfg["eng"]["rcp"] == "v":
                nc.vector.reciprocal(out=rn[:, :, :], in_=rn[:, :, :])
            else:
                nc.gpsimd.reciprocal(out=rn[:, :, :], in_=rn[:, :, :])
            o = io.tile([ns, w, 3], f32, tag="o")
            for k in range(3):
                tt(eng["o"], o[:, :, k : k + 1], s[:, :, k : k + 1],
                   rn[:, :, :], Alu.mult)
            nc.sync.dma_start(
                out=oband[r0 : r0 + ns, c0 : c0 + w, :], in_=o[:, :, :]
            )


def _emit_body_v2(nc, io, wk, psp, eps_tile, tid, tmix, vband, oband,
                  grid, in_rows, mybir, cfg):
    """v2: vy comes off the PE as (SH - I) @ V; C2 = cross(d, vy[:, c]).

    Eliminates the vd DRAM double-load and the hd field entirely.
    """
    f32 = mybir.dt.float32
    Alu = mybir.AluOpType
    Act = mybir.ActivationFunctionType
    ENG = {"v": nc.vector, "g": nc.gpsimd}
    eng = {k: ENG[v] for k, v in cfg["eng"].items()}

    def tt(engine, out, in0, in1, op):
        engine.tensor_tensor(out=out, in0=in0, in1=in1, op=op)

    nsp = cfg.get("dma_splits", 1)

    def split_dma(out_tile, in_ap, rows):
        if nsp <= 1:
            nc.sync.dma_start(out=out_tile, in_=in_ap)
            return
        step = (rows + nsp - 1) // nsp
        for p0 in range(0, rows, step):
            p1 = min(p0 + step, rows)
            nc.sync.dma_start(out=out_tile[p0:p1], in_=in_ap[p0:p1])

    units = [(r0, nv, c0, w)
             for r0, nv in _row_blocks(in_rows)
             for c0, w in _col_chunks(grid, cfg["col_chunk"])]
    if cfg.get("interleave", False):
        nchunk = len(_col_chunks(grid, cfg["col_chunk"]))
        blocks = [units[i : i + nchunk] for i in range(0, len(units), nchunk)]
        units = [u for tup in __import__("itertools").zip_longest(*blocks)
                 for u in tup if u is not None]
    for r0, nv, c0, w in units:
        if True:
            nq = nv - 1
            ns = nv - 2
            v = io.tile([nv, w + 2, 3], f32, tag="v")
            split_dma(v[:, :, :], vband[r0 : r0 + nv, c0 : c0 + w + 2, :], nv)

            # vy = (SH - I) @ v on the tensor engine; ACT copies PSUM->SBUF
            vy = wk.tile([nq, w + 2, 3], f32, tag="vy")
            for j0, pw in _col_chunks(w + 2, cfg["psum_cols"]):
                psv = psp.tile([128, pw, 3], f32, tag="psv")
                nc.tensor.matmul(
                    out=psv[:, :, :], lhsT=tmix[0:nv, 1:129],
                    rhs=v[:, j0 : j0 + pw, :], start=True, stop=True,
                )
                nc.scalar.activation(
                    out=vy[:, j0 : j0 + pw, :], in_=psv[0:nq, :, :],
                    func=Act.Copy,
                )

            hx = wk.tile([nq, w + 1, 3], f32, tag="hx")
            tt(eng["hx"], hx[:, :, :], v[0:nq, 1 : w + 2, :],
               v[0:nq, 0 : w + 1, :], Alu.subtract)
            dd = wk.tile([nq, w + 1, 3], f32, tag="dd")
            tt(eng["dd"], dd[:, :, :], hx[:, :, :], vy[:, 1 : w + 2, :], Alu.add)

            # C1 = cross(hx, vy(c+1));  C2 = cross(dd, vy(c))
            m1 = wk.tile([nq, w + 1, 3], f32, tag="m1")
            m2 = wk.tile([nq, w + 1, 3], f32, tag="m2")
            c1 = wk.tile([nq, w + 1, 3], f32, tag="c1")
            c2 = wk.tile([nq, w + 1, 3], f32, tag="c2")
            for k in range(3):
                u, x = (k + 1) % 3, (k + 2) % 3
                tt(eng["mm1"], m1[:, :, k : k + 1], hx[:, :, u : u + 1],
                   vy[:, 1 : w + 2, x : x + 1], Alu.mult)
                tt(eng["mm2"], m2[:, :, k : k + 1], hx[:, :, x : x + 1],
                   vy[:, 1 : w + 2, u : u + 1], Alu.mult)
            tt(eng["c1s"], c1[:, :, :], m1[:, :, :], m2[:, :, :], Alu.subtract)
            for k in range(3):
                u, x = (k + 1) % 3, (k + 2) % 3
                tt(eng["mm1"], m1[:, :, k : k + 1], dd[:, :, u : u + 1],
                   vy[:, 0 : w + 1, x : x + 1], Alu.mult)
                tt(eng["mm2"], m2[:, :, k : k + 1], dd[:, :, x : x + 1],
                   vy[:, 0 : w + 1, u : u + 1], Alu.mult)
            tt(eng["c2s"], c2[:, :, :], m1[:, :, :], m2[:, :, :], Alu.subtract)

            t = wk.tile([nq, w + 1, 3], f32, tag="t")
            tt(eng["t"], t[:, :, :], c1[:, :, :], c2[:, :, :], Alu.add)
            pe_p = cfg.get("pe_p", False)
            if not pe_p:
                p = wk.tile([nq, w, 3], f32, tag="p")
                tt(eng["p"], p[:, :, :], t[:, 1 : w + 1, :], c1[:, 0:w, :],
                   Alu.add)
            q = wk.tile([nq, w, 3], f32, tag="q")
            tt(eng["q"], q[:, :, :], t[:, 0:w, :], c2[:, 1 : w + 1, :], Alu.add)

            s = wk.tile([ns, w, 3], f32, tag="s")
            for j0, pw in _col_chunks(w, cfg["psum_cols"]):
                ps = psp.tile([128, pw, 3], f32, tag="ps")
                if pe_p:
                    nc.tensor.matmul(
                        out=ps[:, :, :], lhsT=tid[0:nq, 1:129],
                        rhs=t[:, 1 + j0 : 1 + j0 + pw, :],
                        start=True, stop=False,
                    )
                    nc.tensor.matmul(
                        out=ps[:, :, :], lhsT=tid[0:nq, 1:129],
                        rhs=c1[:, j0 : j0 + pw, :], start=False, stop=True,
                    )
                else:
                    nc.tensor.matmul(
                        out=ps[:, :, :], lhsT=tid[0:nq, 1:129],
                        rhs=p[:, j0 : j0 + pw, :], start=True, stop=True,
                    )
                tt(eng["s"], s[:, j0 : j0 + pw, :], ps[0:ns, :, :],
                   q[0:ns, j0 : j0 + pw, :], Alu.add)

            sq = wk.tile([ns, w, 3], f32, tag="m1")
            nc.scalar.activation(out=sq[:, :, :], in_=s[:, :, :], func=Act.Square)
            nsq = wk.tile([ns, w, 1], f32, tag="nsq")
            tt(eng["nsq"], nsq[:, :, :], sq[:, :, 0:1], sq[:, :, 1:2], Alu.add)
            tt(eng["nsq"], nsq[:, :, :], nsq[:, :, :], sq[:, :, 2:3], Alu.add)
            rn = wk.tile([ns, w, 1], f32, tag="rn")
            nc.scalar.activation(
                out=rn[:, :, :], in_=nsq[:, :, :], func=Act.Sqrt,
                bias=eps_tile[:ns, :],
            )
            if cfg["eng"]["rcp"] == "v":
                nc.vector.reciprocal(out=rn[:, :, :], in_=rn[:, :, :])
            else:
                nc.gpsimd.reciprocal(out=rn[:, :, :], in_=rn[:, :, :])
            o = io.tile([ns, w, 3], f32, tag="o")
            for k in range(3):
                tt(eng["o"], o[:, :, k : k + 1], s[:, :, k : k + 1],
                   rn[:, :, :], Alu.mult)
            osp = cfg.get("dma_splits_o", 1)
            if osp <= 1:
                nc.sync.dma_start(
                    out=oband[r0 : r0 + ns, c0 : c0 + w, :], in_=o[:, :, :]
                )
            else:
                step = (ns + osp - 1) // osp
                for p0 in range(0, ns, step):
                    p1 = min(p0 + step, ns)
                    nc.sync.dma_start(
                        out=oband[r0 + p0 : r0 + p1, c0 : c0 + w, :],
                        in_=o[p0:p1, :, :],
                    )


_PROGRAM_CACHE: dict = {}


def _get_program(grid: int, n_cores: int, repeats: int = 1, cfg=None):
    cfg = cfg or DEFAULT_CFG
    key = (grid, n_cores, repeats, _cfg_key(cfg))
    if key not in _PROGRAM_CACHE:
        _PROGRAM_CACHE[key] = _build_program(grid, n_cores, repeats, cfg)
    return _PROGRAM_CACHE[key]


def _run_stencil_on_device(vertices: np.ndarray, grid: int, n_cores: int,
                           trace: bool = False, repeats: int = 1, cfg=None):
    from concourse.bass_utils import run_bass_kernel_spmd

    base, out_rows, in_rows = _band_layout(grid, n_cores)
    V = np.ascontiguousarray(vertices.reshape(grid, grid, 3).astype(np.float32))
    VP = np.pad(V, ((1, 1), (1, 1), (0, 0)), mode="edge")

    in_maps = [
        {"vband": np.ascontiguousarray(VP[base * k : base * k + in_rows])}
        for k in range(n_cores)
    ]
    nc = _get_program(grid, n_cores, repeats, cfg)
    kres = run_bass_kernel_spmd(nc, in_maps, list(range(n_cores)), trace=trace)

    out = np.empty((grid, grid, 3), dtype=np.float32)
    for k in range(n_cores):
        ob = kres.results[k]["oband"]
        take = out_rows - 1 if k < n_cores - 1 else out_rows
        out[base * k : base * k + take] = ob[:take]
    return out.reshape(grid * grid, 3), kres


def kernel(vertices: np.ndarray, faces: np.ndarray) -> np.ndarray:
    vertices = np.asarray(vertices, dtype=np.float32)
    faces = np.asarray(faces)
    grid = int(round(np.sqrt(vertices.shape[0])))
    if (
        grid * grid == vertices.shape[0]
        and (grid - 1) % N_CORES == 0
        and _is_structured(faces, grid)
    ):
        out, _ = _run_stencil_on_device(vertices, grid, N_CORES)
        return out
    print("kernel: faces are not the structured triangulation; host fallback",
          file=sys.stderr)
    return _host_fallback(vertices, faces)



# revision 3
# speedup vs baseline: 3.5592x; 3.5592x over previous
"""Mesh vertex-normals kernel v3: SoA fp16 planes, folded row bands.

Differences vs v2 (kernel.py):
  * fp16 I/O and compute: DVE TensorTensor gets the 2x packed mode
    (2-byte dtype + unit-stride last dim); DMA bytes halve.
  * SoA on the free axis: tiles are [rows, 3, cols] so every op —
    including the 12 cross-product component mults — is unit-stride.
  * Folded band: the leftover 58-row block is folded into 2 column
    strips stacked on the partition axis, so its per-partition cost
    halves (116 busy partitions instead of 58).
  * Row shift for S = shift_down(P) + Q runs as an SBUF->SBUF DMA
    (partition-offset copy) instead of a PE matmul.
  * Normalization: ACT Square -> adds -> ACT Sqrt(+eps) -> DVE divide.

Host side converts vertices to fp16 [rows, 3, cols] planes (pad +
transpose) and converts the fp16 output planes back to f32.
"""

import sys

sys.path.insert(0, "/opt/trn_rl_repo")

import numpy as np

GRID = 1449
N_CORES = 8


# ---------------------------------------------------------------------------
# host-side helpers
# ---------------------------------------------------------------------------

def _is_structured(faces: np.ndarray, grid: int) -> bool:
    n_quads = (grid - 1) * (grid - 1)
    if faces.shape != (2 * n_quads, 3):
        return False
    idx = np.arange(grid * grid, dtype=np.int64).reshape(grid, grid)
    i00 = idx[:-1, :-1].ravel()
    i01 = idx[:-1, 1:].ravel()
    i10 = idx[1:, :-1].ravel()
    i11 = idx[1:, 1:].ravel()
    f = faces
    return (
        np.array_equal(f[:n_quads, 0], i00)
        and np.array_equal(f[:n_quads, 1], i01)
        and np.array_equal(f[:n_quads, 2], i11)
        and np.array_equal(f[n_quads:, 0], i00)
        and np.array_equal(f[n_quads:, 1], i11)
        and np.array_equal(f[n_quads:, 2], i10)
    )


def _host_fallback(vertices: np.ndarray, faces: np.ndarray) -> np.ndarray:
    n_vertices = vertices.shape[0]
    va = vertices[faces[:, 0]]
    vb = vertices[faces[:, 1]]
    vc = vertices[faces[:, 2]]
    cross = np.cross(vb - va, vc - vb).astype(np.float32)
    norm = np.linalg.norm(cross, axis=-1, keepdims=True)
    weighted = (cross / norm) * (norm * 0.5)
    data = np.broadcast_to(weighted[:, None, :], (faces.shape[0], 3, 3)).reshape(-1, 3)
    summed = np.zeros((n_vertices, 3), dtype=np.float32)
    np.add.at(summed, faces.reshape(-1), data)
    norms = np.linalg.norm(summed, axis=-1, keepdims=True)
    return (summed / np.maximum(norms, 1e-10)).astype(np.float32)


def _band_layout(grid: int, n_cores: int):
    base = (grid - 1) // n_cores
    assert base * n_cores == grid - 1, "grid-1 must divide evenly"
    out_rows = base + 1
    in_rows = base + 3
    return base, out_rows, in_rows


def _col_chunks(width: int, chunk: int):
    return [(c0, min(chunk, width - c0)) for c0 in range(0, width, chunk)]


def _overlap_chunks(total: int, n: int):
    """n equal-width chunks covering [0, total); later chunks may overlap
    earlier ones. Yields (c0, so, wst): load cols c0..c0+w, store local
    cols so..so+wst to grid cols c0+so..c0+so+wst. All widths equal w."""
    w = -(-total // n)
    out = []
    for j in range(n):
        store_start = j * w
        store_end = min((j + 1) * w, total)
        c0 = min(j * w, total - w)
        out.append((c0, store_start - c0, store_end - store_start))
    return w, out


def _fold_units(grid: int, n_cores: int, chunks_a: int, chunks_b: int = 1):
    """Units: each = dict(P, w, rects=[(p0, nv, r0, c0, so, wst)]).

    Rect semantics: partitions p0..p0+nv hold padded-band v-rows
    r0..r0+nv; loads fetch w+2 cols from c0; stores write local cols
    so..so+wst to grid cols c0+so..
    """
    base, out_rows, in_rows = _band_layout(grid, n_cores)
    units = []
    if in_rows <= 128:
        w, chunks = _overlap_chunks(grid, chunks_a)
        for c0, so, wst in chunks:
            units.append(dict(P=in_rows, w=w,
                              rects=[(0, in_rows, 0, c0, so, wst)]))
        return units
    # 128-row rect A + leftover rect B folded into column strips
    nv_b = in_rows - 126
    assert nv_b >= 3
    w, chunks = _overlap_chunks(grid, chunks_a)
    for c0, so, wst in chunks:
        units.append(dict(P=128, w=w, rects=[(0, 128, 0, c0, so, wst)]))
    nstrips = 128 // nv_b
    wb, bstrips = _overlap_chunks(grid, nstrips)
    # chunk each strip's columns as well, so B tiles stay small
    wbc, bcols = _overlap_chunks(wb, chunks_b)
    for (coff, so2, wst2) in bcols:
        ch_lo, ch_hi = coff + so2, coff + so2 + wst2
        rects = []
        for j, (c0s, so, wst) in enumerate(bstrips):
            # intersect the strip's store range [so, so+wst) with the
            # column chunk's store range, both in strip-local coords
            lo = max(so, ch_lo)
            hi = min(so + wst, ch_hi)
            if hi <= lo:
                continue
            rects.append((j * nv_b, nv_b, 126, c0s + coff, lo - coff, hi - lo))
        units.append(dict(P=nstrips * nv_b, w=wbc, rects=rects))
    return units


# ---------------------------------------------------------------------------
# device program
# ---------------------------------------------------------------------------

DEFAULT_CFG = dict(
    chunks_a=5,
    chunks_b=3,
    sw_pipe=1,
    io_bufs=3,
    wk_bufs=4,
    psum_bufs=4,
    psum_cols=162,     # cols per PSUM chunk (x3 planes x4B <= 2KB)
    sq="act",          # 'act' | 'v' | 'g'
    o_bcast=True,      # single broadcast divide vs 3 per-plane divides
    vyf_pe=True,       # vyf = (SH - I) @ v on the tensor engine
    pshift="pe",       # 'pe' (matmul into PSUM) | 'dma' (SBUF shift copy)
    # engine per op: 'v' = vector (DVE), 'g' = gpsimd (Pool)
    eng=dict(vyf="v", hx="g", dd="g", mm1="v", mm2="g", c1="g", c2="g",
             t="g", p="g", q="v", s="v", nsq="v", o="v"),
    st_eng="act",
)


def _cfg_key(cfg):
    e = cfg["eng"]
    return (cfg["chunks_a"], cfg.get("chunks_b", 1),
            cfg["io_bufs"], cfg["wk_bufs"], cfg["sq"],
            cfg.get("o_bcast", True), cfg.get("vyf_pe", False),
            cfg.get("pshift", "dma"), cfg.get("psum_cols", 162),
            cfg.get("psum_bufs", 4), cfg.get("st_eng", "act"),
            cfg.get("s_f16", True), tuple(sorted(e.items())))


def _build_program(grid: int, n_cores: int, repeats: int = 1, cfg=None):
    import contextlib

    import concourse.bacc as bacc
    import concourse.tile as tile
    from concourse import mybir

    cfg = cfg or DEFAULT_CFG
    f16 = mybir.dt.float16
    f32 = mybir.dt.float32

    base, out_rows, in_rows = _band_layout(grid, n_cores)
    W = grid + 2

    nc = bacc.Bacc()
    vband = nc.dram_tensor("vband", [in_rows, 3, W], f32, kind="ExternalInput")
    oband = nc.dram_tensor("oband", [out_rows, 3, grid], f16,
                           kind="ExternalOutput")

    units = _fold_units(grid, n_cores, cfg["chunks_a"], cfg.get("chunks_b", 1))
    for i, u in enumerate(units):
        u["idx"] = i

    with tile.TileContext(nc) as tc:
        with (
            tc.tile_pool(name="io", bufs=cfg["io_bufs"]) as io,
            tc.tile_pool(name="wk", bufs=cfg["wk_bufs"]) as wk,
            tc.tile_pool(name="ps", bufs=cfg.get("psum_bufs", 4),
                         space="PSUM") as psp,
            tc.tile_pool(name="cst", bufs=1) as cst,
        ):
            from concourse.masks import make_identity

            eps_tile = cst.tile([128, 1], f16, tag="eps")
            nc.vector.memset(eps_tile[:, :], 1e-7)
            # tid[:, 1:129] = down-shift matrix SH[k, m] = 1 iff k == m+1
            tid = cst.tile([128, 130], f32, tag="tid")
            nc.gpsimd.memset(tid[:, :], 0.0)
            make_identity(nc, tid[:, 0:128], nomemset=True)
            # tmix[:, 1:129][k, m] = +1 if k == m+1 else (-1 if k == m)
            tmix = cst.tile([128, 130], f32, tag="tmix")
            nc.gpsimd.memset(tmix[:, :], 0.0)
            make_identity(nc, tmix[:, 0:128], nomemset=True)
            nc.gpsimd.affine_select(
                out=tmix[:, 1:129], in_=tmix[:, 1:129],
                compare_op=mybir.AluOpType.not_equal, fill=-1.0, base=0,
                pattern=[[-1, 128]], channel_multiplier=1,
            )

            loop = tc.For_i(0, repeats, 1) if repeats > 1 else contextlib.nullcontext()
            with loop:
                stages = [
                    _emit_unit(nc, io, wk, psp, eps_tile, tid, tmix, unit,
                               vband, oband, mybir, cfg)
                    for unit in units
                ]
                skew = cfg.get("sw_pipe", 0)
                if skew:
                    # software pipeline: stage k of unit u emits at step
                    # u + k*skew; later stages (older units) first, so each
                    # engine's queue interleaves independent units.
                    nst = len(stages[0])
                    total = len(units) + (nst - 1) * skew
                    for step in range(total):
                        for stg in range(nst - 1, -1, -1):
                            ui = step - stg * skew
                            if 0 <= ui < len(units):
                                stages[ui][stg]()
                else:
                    for fs in stages:
                        for f in fs:
                            f()

    nc.finalize()
    return nc


def _psum_chunks(width: int, chunk: int):
    return [(j0, min(chunk, width - j0)) for j0 in range(0, width, chunk)]


def _emit_unit(nc, io, wk, psp, eps_tile, tid, tmix, unit, vband, oband,
               mybir, cfg):
    """Returns a list of stage closures: [load, vyf, crosses, stencil, norm].

    Calling them in order emits the unit; a software-pipelined caller can
    interleave stages of different units.
    """
    f16 = mybir.dt.float16
    f32 = mybir.dt.float32
    Alu = mybir.AluOpType
    Act = mybir.ActivationFunctionType
    ui = unit.get("idx", 0)
    ENG = {"v": nc.vector, "g": nc.gpsimd,
           "a": nc.vector if ui % 2 == 0 else nc.gpsimd,
           "b": nc.gpsimd if ui % 2 == 0 else nc.vector}
    eng = {k: ENG[v] for k, v in cfg["eng"].items()}

    def tt(tag, out, in0, in1, op):
        eng[tag].tensor_tensor(out=out, in0=in0, in1=in1, op=op)

    P, w, rects = unit["P"], unit["w"], unit["rects"]
    w2 = w + 2
    in_rows = vband.shape[0]
    pcols = cfg.get("psum_cols", 162)
    ts = {}  # tiles shared across stages

    def stage_load():
        v = ts["v"] = io.tile([P, 3, w2], f32, tag="v", name="v")
        for (p0, nv, r0, c0, so, wst) in rects:
            nc.sync.dma_start(out=v[p0:p0 + nv, :, :],
                              in_=vband[r0:r0 + nv, :, c0:c0 + w2])
        if not cfg.get("vyf_pe", False):
            vdn = ts["vdn"] = io.tile([P, 3, w2], f32, tag="vdn", name="vdn")
            for (p0, nv, r0, c0, so, wst) in rects:
                # duplicate the band's last row if the shifted window runs
                # off the end (that partition is never used)
                n_load = min(nv, in_rows - (r0 + 1))
                nc.sync.dma_start(
                    out=vdn[p0:p0 + n_load, :, :],
                    in_=vband[r0 + 1:r0 + 1 + n_load, :, c0:c0 + w2])
                if n_load < nv:
                    nc.sync.dma_start(
                        out=vdn[p0 + nv - 1:p0 + nv, :, :],
                        in_=vband[in_rows - 1:in_rows, :, c0:c0 + w2])

    def stage_vyf():
        v = ts["v"]
        vyf = ts["vyf"] = wk.tile([P, 3, w2], f32, tag="vyf", name="vyf")
        if cfg.get("vyf_pe", False):
            # vyf = (SH - I) @ v on the tensor engine; ACT copies PSUM out.
            # Seam partitions mix adjacent rects; they are never consumed.
            for j0, pw in _psum_chunks(w2, pcols):
                psv = psp.tile([128, 3, pw], f32, tag="psv", name="psv")
                nc.tensor.matmul(out=psv[:, :, :], lhsT=tmix[0:P, 1:129],
                                 rhs=v[:, :, j0:j0 + pw], start=True, stop=True)
                nc.scalar.activation(out=vyf[:, :, j0:j0 + pw],
                                     in_=psv[0:P, :, :], func=Act.Copy)
        else:
            tt("vyf", vyf[:, :, :], ts["vdn"][:, :, :], v[:, :, :],
               Alu.subtract)
        hx = ts["hx"] = wk.tile([P, 3, w + 1], f32, tag="hx", name="hx")
        tt("hx", hx[:, :, :], v[:, :, 1:w2], v[:, :, 0:w + 1], Alu.subtract)

    def stage_cross():
        vyf, hx = ts["vyf"], ts["hx"]
        dd = ts["dd"] = wk.tile([P, 3, w + 1], f32, tag="dd", name="dd")
        tt("dd", dd[:, :, :], hx[:, :, :], vyf[:, :, 1:w2], Alu.add)
        m1 = wk.tile([P, 3, w + 1], f32, tag="m1", name="m1")
        m2 = wk.tile([P, 3, w + 1], f32, tag="m2", name="m2")
        c1 = ts["c1"] = wk.tile([P, 3, w + 1], f32, tag="c1", name="c1")
        for k in range(3):
            u, x = (k + 1) % 3, (k + 2) % 3
            tt("mm1", m1[:, k:k + 1, :], hx[:, u:u + 1, :],
               vyf[:, x:x + 1, 1:w2], Alu.mult)
            tt("mm2", m2[:, k:k + 1, :], hx[:, x:x + 1, :],
               vyf[:, u:u + 1, 1:w2], Alu.mult)
        tt("c1", c1[:, :, :], m1[:, :, :], m2[:, :, :], Alu.subtract)
        m3 = wk.tile([P, 3, w + 1], f32, tag="m1", name="m3")
        m4 = wk.tile([P, 3, w + 1], f32, tag="m2", name="m4")
        c2 = ts["c2"] = wk.tile([P, 3, w + 1], f32, tag="c2", name="c2")
        for k in range(3):
            u, x = (k + 1) % 3, (k + 2) % 3
            tt("mm1", m3[:, k:k + 1, :], dd[:, u:u + 1, :],
               vyf[:, x:x + 1, 0:w + 1], Alu.mult)
            tt("mm2", m4[:, k:k + 1, :], dd[:, x:x + 1, :],
               vyf[:, u:u + 1, 0:w + 1], Alu.mult)
        tt("c2", c2[:, :, :], m3[:, :, :], m4[:, :, :], Alu.subtract)

    def stage_stencil():
        c1, c2 = ts["c1"], ts["c2"]
        # T = C1+C2; P = T(c+1)+C1; Q = T+C2(c+1); S = down(P)+Q
        t = wk.tile([P, 3, w + 1], f32, tag="t", name="t")
        tt("t", t[:, :, :], c1[:, :, :], c2[:, :, :], Alu.add)
        p = wk.tile([P, 3, w], f32, tag="dd", name="p")
        tt("p", p[:, :, :], t[:, :, 1:w + 1], c1[:, :, 0:w], Alu.add)
        q = wk.tile([P, 3, w], f32, tag="q", name="q")
        tt("q", q[:, :, :], t[:, :, 0:w], c2[:, :, 1:w + 1], Alu.add)

        sdt = f16 if cfg.get("s_f16", True) else f32
        s = ts["s"] = wk.tile([P, 3, w], sdt, tag="hx", name="s")
        if cfg.get("pshift", "dma") == "pe":
            # s = SH @ p + q: the shift runs on the tensor engine into PSUM
            for j0, pw in _psum_chunks(w, pcols):
                pss = psp.tile([128, 3, pw], f32, tag="pss", name="pss")
                nc.tensor.matmul(out=pss[:, :, :], lhsT=tid[0:P, 1:129],
                                 rhs=p[:, :, j0:j0 + pw], start=True,
                                 stop=True)
                tt("s", s[:, :, j0:j0 + pw], pss[0:P, :, :],
                   q[:, :, j0:j0 + pw], Alu.add)
            ts["Q"] = P
        else:
            # full-tile partition shift; seam partitions get cross-rect
            # garbage, which post-shift ops compute on but stores never read
            pdn = wk.tile([P, 3, w], f32, tag="vyf", name="pdn")
            nc.sync.dma_start(out=pdn[0:P - 1, :, :], in_=p[1:P, :, :])
            ts["Q"] = P - 1
            tt("s", s[0:P - 1, :, :], pdn[0:P - 1, :, :], q[0:P - 1, :, :],
               Alu.add)

    def stage_norm():
        s, Q = ts["s"], ts["Q"]
        sq = wk.tile([P, 3, w], f16, tag="m1", name="sq")
        if cfg["sq"] == "act":
            nc.scalar.activation(out=sq[0:Q, :, :], in_=s[0:Q, :, :],
                                 func=Act.Square)
        else:
            ENG[cfg["sq"]].tensor_tensor(out=sq[0:Q, :, :], in0=s[0:Q, :, :],
                                         in1=s[0:Q, :, :], op=Alu.mult)
        nsq = wk.tile([P, 1, w], f16, tag="nsq", name="nsq")
        tt("nsq", nsq[0:Q, :, :], sq[0:Q, 0:1, :], sq[0:Q, 1:2, :], Alu.add)
        tt("nsq", nsq[0:Q, :, :], nsq[0:Q, :, :], sq[0:Q, 2:3, :], Alu.add)
        rn = wk.tile([P, 1, w], f16, tag="rn", name="rn")
        nc.scalar.activation(out=rn[0:Q, :, :], in_=nsq[0:Q, :, :],
                             func=Act.Sqrt, bias=eps_tile[:Q, :])
        with nc.allow_low_precision(reason="1/norm fine in fp16"):
            if cfg["eng"].get("rcp", "v") == "v":
                nc.vector.reciprocal(out=rn[0:Q, :, :], in_=rn[0:Q, :, :])
            else:
                nc.gpsimd.reciprocal(out=rn[0:Q, :, :], in_=rn[0:Q, :, :])
        o = io.tile([P, 3, w], f16, tag="o", name="o")
        if cfg.get("o_bcast", True):
            tt("o", o[0:Q, :, :], s[0:Q, :, :],
               rn[0:Q, :, :].broadcast_to((Q, 3, w)), Alu.mult)
        else:
            for k in range(3):
                tt("o", o[0:Q, k:k + 1, :], s[0:Q, k:k + 1, :],
                   rn[0:Q, :, :], Alu.mult)
        st = {"sp": nc.sync, "act": nc.scalar,
              "g": nc.gpsimd}[cfg.get("st_eng", "act")]
        for (p0, nv, r0, c0, so, wst) in rects:
            ns = nv - 2
            st.dma_start(out=oband[r0:r0 + ns, :, c0 + so:c0 + so + wst],
                         in_=o[p0:p0 + ns, :, so:so + wst])

    return [stage_load, stage_vyf, stage_cross, stage_stencil, stage_norm]


_PROGRAM_CACHE: dict = {}


def _get_program(grid: int, n_cores: int, repeats: int = 1, cfg=None):
    cfg = cfg or DEFAULT_CFG
    key = (grid, n_cores, repeats, _cfg_key(cfg))
    if key not in _PROGRAM_CACHE:
        _PROGRAM_CACHE[key] = _build_program(grid, n_cores, repeats, cfg)
    return _PROGRAM_CACHE[key]


def _make_in_maps(vertices: np.ndarray, grid: int, n_cores: int):
    base, out_rows, in_rows = _band_layout(grid, n_cores)
    V = vertices.reshape(grid, grid, 3)
    VP = np.pad(V, ((1, 1), (1, 1), (0, 0)), mode="edge")
    VPT = np.ascontiguousarray(VP.transpose(0, 2, 1))
    return [
        {"vband": np.ascontiguousarray(VPT[base * k: base * k + in_rows])}
        for k in range(n_cores)
    ]


def _assemble_out(results, grid: int, n_cores: int) -> np.ndarray:
    base, out_rows, in_rows = _band_layout(grid, n_cores)
    out = np.empty((grid, grid, 3), dtype=np.float32)
    for k in range(n_cores):
        ob = results[k]["oband"]  # [out_rows, 3, grid] f16
        take = out_rows - 1 if k < n_cores - 1 else out_rows
        out[base * k: base * k + take] = (
            ob[:take].transpose(0, 2, 1).astype(np.float32)
        )
    return out.reshape(grid * grid, 3)


def _run_stencil_on_device(vertices: np.ndarray, grid: int, n_cores: int,
                           trace: bool = False, repeats: int = 1, cfg=None):
    from concourse.bass_utils import run_bass_kernel_spmd

    in_maps = _make_in_maps(vertices, grid, n_cores)
    nc = _get_program(grid, n_cores, repeats, cfg)
    kres = run_bass_kernel_spmd(nc, in_maps, list(range(n_cores)), trace=trace)
    return _assemble_out(kres.results, grid, n_cores), kres


def kernel(vertices: np.ndarray, faces: np.ndarray) -> np.ndarray:
    vertices = np.asarray(vertices, dtype=np.float32)
    faces = np.asarray(faces)
    grid = int(round(np.sqrt(vertices.shape[0])))
    if (
        grid * grid == vertices.shape[0]
        and (grid - 1) % N_CORES == 0
        and _is_structured(faces, grid)
    ):
        out, _ = _run_stencil_on_device(vertices, grid, N_CORES)
        return out
    print("kernel: faces are not the structured triangulation; host fallback",
          file=sys.stderr)
    return _host_fallback(vertices, faces)
